# revision 27
# baseline (speedup 1.0000x reference)
"""Bass/Trainium2 kernel for nn_Attention (general-score cross-attention softmax).

Reference math:
    proj[s,b,k]  = sum_h e[s,b,h] * W[k,h] + bias[k]
    scores[b,s]  = sum_k hidden[b,k] * proj[s,b,k]
    out[b,0,s]   = softmax_s(scores[b,s])

Algebraic rewrite:
    scores[b,s] = sum_h g[b,h] * e[s,b,h] + (hidden[b] . bias)
with g = hidden[0] @ W. The per-b constant cancels under softmax (shift
invariance), so bias never enters. This removes the S*B*H*H matmul; what
remains is a batched matvec over encoder_outputs plus a softmax.

Device strategy: the host pre-transposes each core's e-slice to [b, h, s]
(fp16) so the contraction axis h lands on SBUF partitions. The TensorEngine
then does the whole matvec as [K=128, M=1, N=512] matmuls accumulating over
h-chunks in PSUM (f32). Scores land directly in [b, s] layout; a short f32
softmax finishes on-chip. VectorE/ScalarE see only KB-scale traffic, so the
kernel is purely DMA-bound on the 16 MB fp16 stream.

Sharding: data-parallel over batch, 8 cores x 4 batches, no collectives;
the host concatenates the per-core [4, 2048] outputs.
"""

import sys

import numpy as np

sys.path.insert(0, "/opt/trn_rl_repo")

from concourse import bacc, mybir, tile  # noqa: E402
from concourse.bass_utils import run_bass_kernel_spmd  # noqa: E402

F32 = mybir.dt.float32
F16 = mybir.dt.float16
NCORES = 8
S, B, H = 2048, 32, 1024
BL = B // NCORES   # 4 batches per core
KP = 128           # contraction partitions per matmul
NK = H // KP       # 8 h-chunks
NC_ = 512          # matmul N (one PSUM bank of f32)
NCH = S // NC_     # 4 s-chunks
NTILES = BL * NK   # 32 streamed tiles of [128, 2048] fp16

_NC_CACHE = None


def _build_nc():
    nc = bacc.Bacc("TRN2", target_bir_lowering=False, debug=False,
                   num_devices=NCORES)
    # enc[j2, p, half*S + s] = e[s, b, (2*kp+half)*128 + p] (fp16),
    # j2 = b*(NK//2) + kp — 1 MB DMAs carrying two h-chunks each.
    enc = nc.dram_tensor("enc", [NTILES // 2, KP, 2 * S], F16,
                         kind="ExternalInput")
    # gt[p, j] = g[b, k*128 + p] (fp16), j = b*NK + k
    gt = nc.dram_tensor("gt", [KP, NTILES], F16, kind="ExternalInput")
    out = nc.dram_tensor("out", [BL, S], F32, kind="ExternalOutput")

    with tile.TileContext(nc) as tc:
        with tc.tile_pool(name="consts", bufs=1) as consts, \
             tc.tile_pool(name="io", bufs=6) as io, \
             tc.tile_pool(name="ps", bufs=2, space="PSUM") as psum:
            gt_t = consts.tile([KP, NTILES], F16)
            nc.scalar.dma_start(out=gt_t[:], in_=gt[:])

            # All scores live on partition 0 (engine APs must start at
            # quad-aligned partitions); [b, s] rows form in the final DMA.
            scb = consts.tile([1, BL * S], F32)
            colmax = consts.tile([1, BL * NCH], F32)
            pexp = consts.tile([1, BL * S], F32)
            scbn = consts.tile([1, BL * S], F32)
            negm = consts.tile([1, BL], F32)
            ssum = consts.tile([1, BL], F32)
            rs = consts.tile([1, BL], F32)

            # The Sync ring carries ONLY the streaming et DMAs. Anything
            # paced by the softmax chain (gt, per-b outputs) issues from the
            # Scalar ring instead — otherwise those DMA issues queue behind
            # evac/exp in ACT program order and stall the stream (observed
            # as 1.4-4.4 us matmul stalls per batch).
            for b in range(BL):
                psg = [psum.tile([1, NC_], F32, tag=f"psg{c}",
                                 name=f"psg{b}_{c}")
                       for c in range(NCH)]
                for kp in range(NK // 2):
                    et = io.tile([KP, 2 * S], F16, tag="et")
                    nc.sync.dma_start(
                        out=et[:], in_=enc[b * (NK // 2) + kp])
                    for half in range(2):
                        j = b * NK + kp * 2 + half
                        for c in range(NCH):
                            nc.tensor.matmul(
                                psg[c][:],
                                gt_t[:, j:j + 1],
                                et[:, half * S + c * NC_:
                                   half * S + (c + 1) * NC_],
                                start=(kp == 0 and half == 0),
                                stop=(kp == NK // 2 - 1 and half == 1),
                            )
                # Per-b softmax, overlapped with the next b's streaming.
                # colmax reads PSUM directly so DVE runs while ACT evacuates.
                for c in range(NCH):
                    off = b * S + c * NC_
                    nc.vector.tensor_reduce(
                        out=colmax[0:1, b * NCH + c:b * NCH + c + 1],
                        in_=psg[c][:],
                        axis=mybir.AxisListType.X,
                        op=mybir.AluOpType.max,
                    )
                    nc.scalar.copy(scb[0:1, off:off + NC_], psg[c][:])
                nc.vector.tensor_reduce(
                    out=negm[0:1, b:b + 1],
                    in_=colmax[0:1, b * NCH:(b + 1) * NCH],
                    axis=mybir.AxisListType.X,
                    op=mybir.AluOpType.max, negate=True,
                )
                nc.scalar.activation(
                    out=pexp[0:1, b * S:(b + 1) * S],
                    in_=scb[0:1, b * S:(b + 1) * S],
                    func=mybir.ActivationFunctionType.Exp,
                    bias=negm[0:1, b:b + 1], scale=1.0,
                    accum_out=ssum[0:1, b:b + 1],
                )
                nc.vector.reciprocal(rs[0:1, b:b + 1], ssum[0:1, b:b + 1])
                nc.vector.tensor_scalar_mul(
                    scbn[0:1, b * S:(b + 1) * S],
                    pexp[0:1, b * S:(b + 1) * S],
                    rs[0:1, b:b + 1],
                )
                # Ship each b as soon as it is normalized; only b=BL-1's
                # store sits on the critical tail.
                nc.scalar.dma_start(
                    out=out[b:b + 1, :],
                    in_=scbn[0:1, b * S:(b + 1) * S],
                )

    nc.compile()
    return nc


def _get_nc():
    global _NC_CACHE
    if _NC_CACHE is None:
        _NC_CACHE = _build_nc()
    return _NC_CACHE


def make_in_maps(hidden, encoder_outputs, W, b=None):
    hidden = np.asarray(hidden, dtype=np.float32)
    e = np.asarray(encoder_outputs, dtype=np.float32)
    W = np.asarray(W, dtype=np.float32)
    g = hidden[0] @ W  # [B, H]: g[b,h] = sum_k hidden[b,k] W[k,h]
    e16 = e.astype(np.float16)
    g16 = g.astype(np.float16)
    in_maps = []
    for c in range(NCORES):
        bs = slice(c * BL, (c + 1) * BL)
        # [S, BL, H] -> [BL, H, S] -> [BL, NK/2, 2, KP, S] -> pair-interleave
        enc_c = np.ascontiguousarray(
            e16[:, bs, :].transpose(1, 2, 0)
            .reshape(BL, NK // 2, 2, KP, S)
            .transpose(0, 1, 3, 2, 4)
        ).reshape(NTILES // 2, KP, 2 * S)
        # gt[p, b*NK+k] = g[b, k*128+p]
        gt_c = np.ascontiguousarray(
            g16[bs].reshape(BL, NK, KP).transpose(2, 0, 1).reshape(KP, NTILES)
        )
        in_maps.append({"enc": enc_c, "gt": gt_c})
    return in_maps


def kernel(hidden, encoder_outputs, W, b):
    in_maps = make_in_maps(hidden, encoder_outputs, W, b)
    nc = _get_nc()
    res = run_bass_kernel_spmd(nc, in_maps, core_ids=list(range(NCORES)))
    outs = [np.asarray(res.results[c]["out"]).reshape(BL, 1, S)
            for c in range(NCORES)]
    return np.concatenate(outs, axis=0)
